# revision 4
# baseline (speedup 1.0000x reference)
"""Distributed Bass kernel for additive (Bahdanau-style) attention on 8 TRN2
NeuronCores.

Math (reference):
    temp_enc = enc @ Wenc^T                    [1,S,H]
    temp_dec = dec @ Wdec^T                    [1,H]
    x        = tanh(temp_dec + temp_enc)
    e        = x @ va^T                        [1,S,1]
    w        = softmax(mask ? e : -inf)        [1,S,1]
    ctx      = w^T @ enc                       [1,1,H]

Strategy: shard S across 8 cores (8192 rows each). Each core:
  - streams its enc shard from HBM once (f32), casts to bf16,
    transposes s-h via the DMA xbar (2-byte transpose, no PE/PSUM cost),
  - temp_enc^T = WencT @ encT on TensorE (bf16, f32 accum in PSUM),
  - tanh+bias fused on ScalarE (bias = temp_dec as per-partition scalar,
    since the layout is transposed), output bf16,
  - energies via TensorE with the tanh block stationary and va moving
    (output lands as [128,1] columns -> natural [128,64] softmax layout),
  - exp WITHOUT max subtraction (|e| <= ||va||_1 ~ 18, safely in f32),
    masked, then partial context = sum_s exp(e_s) * enc_s accumulated in
    PSUM across all 64 s-tiles,
  - ONE AllReduce(add) of [partial_ctx(512) | partial_Z(1)] (2KB),
  - normalize: ctx = y/Z, w_s = exp(e_s)/Z.

Exactness vs reference: softmax(e) is invariant to the max subtraction up to
f32 rounding; bf16 is only used for matmul operands (f32 accumulation).
"""

import numpy as np

import concourse.bass as bass
import concourse.mybir as mybir
import concourse.tile as tile
from concourse.bass_utils import run_bass_kernel_spmd

NCORES = 8
H = 512
S = 65536
SLOC = S // NCORES  # 8192
P = 128
NST = SLOC // P  # 64 s-tiles of 128
CH = 4  # s-tiles per chunk (chunk = 512 sequence positions)
NCHUNK = NST // CH  # 16
NH = H // P  # 4 h-tiles
AR_N = 520  # 512 ctx + 1 Z + 7 pad (32B-aligned total)

F32 = mybir.dt.float32
BF16 = mybir.dt.bfloat16
AF = mybir.ActivationFunctionType

# ---------------------------------------------------------------------------
# walrus on this container rejects >1 sync-wait on Drain-class instructions
# and >2 on everything else; hoist the excess onto same-engine NOPs.
_LIMITS = {
    "InstDrain": 1,
    "InstNoOp": 1,
    "InstAllEngineBarrier": 1,
    "InstEventSemaphore": 1,
    "InstDmaTransposeAnt": 1,
}
_DEFAULT_LIMIT = 1


def _fix_wait_overflow(nc):
    eng_map = {
        mybir.EngineType.PE: nc.tensor,
        mybir.EngineType.DVE: nc.vector,
        mybir.EngineType.Activation: nc.scalar,
        mybir.EngineType.Pool: nc.gpsimd,
        mybir.EngineType.SP: nc.sync,
    }
    for bb in nc.main_func.blocks:
        insts = bb.instructions
        i = 0
        while i < len(insts):
            ins = insts[i]
            limit = _LIMITS.get(type(ins).__name__, _DEFAULT_LIMIT)
            si = ins.sync_info
            waits = list(si.on_wait) if si and si.on_wait else []
            if len(waits) > limit:
                keep, extra = waits[:limit], waits[limit:]
                eng = eng_map[ins.engine]
                nops = []
                for j in range(len(extra)):
                    nop_ins = eng.nop(nofuse=True).ins
                    nop_ins.sync_info = type(si)(
                        on_wait=[extra[j]], on_update=[]
                    )
                    for b2 in nc.main_func.blocks:
                        if nop_ins in b2.instructions:
                            b2.instructions.remove(nop_ins)
                            break
                    nops.append(nop_ins)
                ins.sync_info = type(si)(on_wait=keep, on_update=si.on_update)
                insts[i:i] = nops
                i += len(nops)
            i += 1
    return nc


# ---------------------------------------------------------------------------
def build_nc():
    nc = bass.Bass()

    enc = nc.declare_dram_parameter("enc", [SLOC, H], F32, isOutput=False)
    dec = nc.declare_dram_parameter("dec", [1, H], F32, isOutput=False)
    msk = nc.declare_dram_parameter("msk", [SLOC], F32, isOutput=False)
    wenc = nc.declare_dram_parameter("wenc", [H, H], F32, isOutput=False)
    wdec = nc.declare_dram_parameter("wdec", [H, H], F32, isOutput=False)
    va = nc.declare_dram_parameter("va", [1, H], F32, isOutput=False)
    attn_out = nc.declare_dram_parameter("attn_out", [SLOC], F32, isOutput=True)
    ctx_out = nc.declare_dram_parameter("ctx_out", [1, H], F32, isOutput=True)

    ar_in = nc.dram_tensor("ar_in", [1, AR_N], F32)
    ar_out = nc.dram_tensor("ar_out", [1, AR_N], F32, addr_space="Shared")

    with tile.TileContext(nc) as tc:
        with (
            tc.tile_pool(name="singles", bufs=1) as singles,
            tc.tile_pool(name="wsetup", bufs=1) as wsetup,
            tc.tile_pool(name="io", bufs=8) as io,
            tc.tile_pool(name="psum1", bufs=4, space="PSUM") as psum1,
            tc.tile_pool(name="psum_e", bufs=2, space="PSUM") as psum_e,
            tc.tile_pool(name="psum_ctx", bufs=1, space="PSUM") as psum_ctx,
            tc.tile_pool(name="psum_tmp", bufs=1, space="PSUM") as psum_tmp,
        ):
            # ---------------- setup: weights ----------------
            # Wenc -> WencT (bf16): load, cast, xbar-transpose
            wencT = []  # [hi] -> [128 h, 512 o] bf16
            wdecT = []  # [hi] -> [128 h, 512 o] bf16
            for name, wdram, dst in (("we", wenc, wencT), ("wd", wdec, wdecT)):
                w_f32 = []
                for oi in range(NH):
                    t = wsetup.tile([P, H], F32, tag=f"{name}f32_{oi}")
                    nc.sync.dma_start(t[:], wdram[oi * P : (oi + 1) * P, :])
                    w_f32.append(t)
                w_bf = []
                for oi in range(NH):
                    t = wsetup.tile([P, H], BF16, tag=f"{name}bf_{oi}")
                    nc.vector.tensor_copy(t[:], w_f32[oi][:])
                    w_bf.append(t)
                for hi in range(NH):
                    t = wsetup.tile([P, H], BF16, tag=f"{name}T_{hi}")
                    dst.append(t)
                for oi in range(NH):
                    for hi in range(NH):
                        nc.sync.dma_start_transpose(
                            dst[hi][:, oi * P : (oi + 1) * P],
                            w_bf[oi][:, hi * P : (hi + 1) * P],
                        )

            # dec / va as per-partition columns [128, NH]
            dec_col_f = singles.tile([P, NH], F32)
            va_col_f = singles.tile([P, NH], F32)
            for hi in range(NH):
                nc.sync.dma_start(
                    dec_col_f[:, hi : hi + 1],
                    dec[0:1, hi * P : (hi + 1) * P].rearrange("a p -> p a"),
                )
                nc.sync.dma_start(
                    va_col_f[:, hi : hi + 1],
                    va[0:1, hi * P : (hi + 1) * P].rearrange("a p -> p a"),
                )
            dec_col = singles.tile([P, NH], BF16)
            va_col = singles.tile([P, NH], BF16)
            nc.vector.tensor_copy(dec_col[:], dec_col_f[:])
            nc.vector.tensor_copy(va_col[:], va_col_f[:])

            # mask as [128, 64] f32, mask_sb[p, j] = mask[j*128 + p]
            mask_sb = singles.tile([P, NST], F32)
            nc.sync.dma_start(mask_sb[:], msk[:].rearrange("(j p) -> p j", p=P))

            ones_col = singles.tile([P, 1], F32)
            nc.vector.memset(ones_col[:], 1.0)
            ones_row = singles.tile([1, P], F32)
            nc.vector.memset(ones_row[:], 1.0)

            # temp_dec^T columns: td[p, oi] = sum_h Wdec[oi*128+p, h] dec[h]
            td_psum = psum_tmp.tile([P, NH], F32, tag="tmp")
            for oi in range(NH):
                for hi in range(NH):
                    nc.tensor.matmul(
                        td_psum[:, oi : oi + 1],
                        wdecT[hi][:, oi * P : (oi + 1) * P],
                        dec_col[:, hi : hi + 1],
                        start=(hi == 0),
                        stop=(hi == NH - 1),
                    )
            td_col = singles.tile([P, NH], F32)
            nc.vector.tensor_copy(td_col[:], td_psum[:])

            # persistent softmax state
            expm = singles.tile([P, NST], F32)  # masked exp(e)
            w_bf16 = singles.tile([P, NST], BF16)  # bf16 copy for ctx matmul
            ctx_psum = psum_ctx.tile([1, H], F32)

            # ---------------- main loop ----------------
            for c in range(NCHUNK):
                s0 = c * CH * P  # global row offset of the chunk
                enc_f32 = []
                for st in range(CH):
                    t = io.tile([P, H], F32, tag="enc_f32")
                    nc.sync.dma_start(
                        t[:], enc[s0 + st * P : s0 + (st + 1) * P, :]
                    )
                    enc_f32.append(t)
                enc_bf = []
                for st in range(CH):
                    t = io.tile([P, H], BF16, tag="enc_bf")
                    nc.vector.tensor_copy(t[:], enc_f32[st][:])
                    enc_bf.append(t)
                # transpose: encT[hi][:, st*128:(st+1)*128] = enc_bf[st][:, hi]^T
                encT = []
                for hi in range(NH):
                    t = io.tile([P, CH * P], BF16, tag="encT")
                    encT.append(t)
                for st in range(CH):
                    for hi in range(NH):
                        nc.sync.dma_start_transpose(
                            encT[hi][:, st * P : (st + 1) * P],
                            enc_bf[st][:, hi * P : (hi + 1) * P],
                        )
                # temp_enc^T blocks + fused tanh
                x_bf = []
                for oi in range(NH):
                    pt = psum1.tile([P, CH * P], F32, tag="psum1")
                    for hi in range(NH):
                        nc.tensor.matmul(
                            pt[:],
                            wencT[hi][:, oi * P : (oi + 1) * P],
                            encT[hi][:],
                            start=(hi == 0),
                            stop=(hi == NH - 1),
                        )
                    xt = io.tile([P, CH * P], BF16, tag="x_bf")
                    nc.scalar.activation(
                        xt[:], pt[:], AF.Tanh, bias=td_col[:, oi : oi + 1]
                    )
                    x_bf.append(xt)
                # energies: e[s] columns into psum_e [128, CH]
                pe_t = psum_e.tile([P, CH], F32, tag="psum_e")
                for st in range(CH):
                    for oi in range(NH):
                        nc.tensor.matmul(
                            pe_t[:, st : st + 1],
                            x_bf[oi][:, st * P : (st + 1) * P],
                            va_col[:, oi : oi + 1],
                            start=(oi == 0),
                            stop=(oi == NH - 1),
                        )
                # exp, mask, bf16 weights
                cols = slice(c * CH, (c + 1) * CH)
                nc.scalar.activation(expm[:, cols], pe_t[:], AF.Exp)
                nc.vector.tensor_mul(expm[:, cols], expm[:, cols], mask_sb[:, cols])
                nc.vector.tensor_copy(w_bf16[:, cols], expm[:, cols])
                # partial context accumulation (k = s on partitions)
                for st in range(CH):
                    j = c * CH + st
                    nc.tensor.matmul(
                        ctx_psum[:],
                        w_bf16[:, j : j + 1],
                        enc_bf[st][:],
                        start=(j == 0),
                        stop=(j == NST - 1),
                        skip_group_check=True,
                    )

            # ---------------- tail ----------------
            zrow = singles.tile([P, 1], F32)
            nc.vector.reduce_sum(zrow[:], expm[:], axis=mybir.AxisListType.X)
            z_psum = psum_tmp.tile([1, 1], F32, tag="tmp")
            nc.tensor.matmul(
                z_psum[:], ones_col[:], zrow[:], start=True, stop=True
            )
            ar_sb = singles.tile([1, AR_N], F32)
            nc.vector.memset(ar_sb[:], 0.0)
            nc.vector.tensor_copy(ar_sb[0:1, 0:H], ctx_psum[:])
            nc.vector.tensor_copy(ar_sb[0:1, H : H + 1], z_psum[:])
            nc.sync.dma_start(ar_in[:], ar_sb[:])
            nc.gpsimd.collective_compute(
                "AllReduce",
                mybir.AluOpType.add,
                replica_groups=[list(range(NCORES))],
                ins=[ar_in[:]],
                outs=[ar_out[:]],
            )
            g = singles.tile([1, AR_N], F32)
            nc.sync.dma_start(g[:], ar_out[:])
            rz = singles.tile([1, 1], F32)
            nc.vector.reciprocal(rz[:], g[0:1, H : H + 1])
            ctx_sb = singles.tile([1, H], F32)
            nc.vector.tensor_scalar_mul(ctx_sb[:], g[0:1, 0:H], rz[:])
            nc.sync.dma_start(ctx_out[:], ctx_sb[:])
            # broadcast 1/Z to all partitions
            b_psum = psum_tmp.tile([P, 1], F32, tag="tmp")
            nc.tensor.matmul(b_psum[:], ones_row[:], rz[:], start=True, stop=True)
            rz_col = singles.tile([P, 1], F32)
            nc.vector.tensor_copy(rz_col[:], b_psum[:])
            w_f32 = singles.tile([P, NST], F32)
            nc.vector.tensor_scalar_mul(w_f32[:], expm[:], rz_col[:])
            nc.sync.dma_start(
                attn_out[:].rearrange("(j p) -> p j", p=P), w_f32[:]
            )

    return _fix_wait_overflow(nc)


_NC = None


def _get_nc():
    global _NC
    if _NC is None:
        _NC = build_nc()
    return _NC


def kernel(encoder_output, decoder_hidden, attention_mask, Wenc_w, Wdec_w, va_w):
    enc = np.ascontiguousarray(np.asarray(encoder_output, dtype=np.float32))[0]
    dec = np.ascontiguousarray(np.asarray(decoder_hidden, dtype=np.float32))
    mask_f = np.asarray(attention_mask).astype(np.float32)
    wenc = np.ascontiguousarray(np.asarray(Wenc_w, dtype=np.float32))
    wdec = np.ascontiguousarray(np.asarray(Wdec_w, dtype=np.float32))
    va = np.ascontiguousarray(np.asarray(va_w, dtype=np.float32))

    in_maps = []
    for r in range(NCORES):
        sl = slice(r * SLOC, (r + 1) * SLOC)
        in_maps.append(
            {
                "enc": np.ascontiguousarray(enc[sl]),
                "dec": dec,
                "msk": np.ascontiguousarray(mask_f[sl]),
                "wenc": wenc,
                "wdec": wdec,
                "va": va,
            }
        )

    res = run_bass_kernel_spmd(_get_nc(), in_maps, list(range(NCORES)))

    attn = np.concatenate(
        [res.results[r]["attn_out"] for r in range(NCORES)]
    ).reshape(1, S, 1)
    ctx = res.results[0]["ctx_out"].reshape(1, 1, H)
    return ctx, attn


# revision 10
# speedup vs baseline: 3.4647x; 3.4647x over previous
"""Distributed Bass kernel for additive (Bahdanau-style) attention on 8 TRN2
NeuronCores.

Math (reference):
    temp_enc = enc @ Wenc^T                    [1,S,H]
    temp_dec = dec @ Wdec^T                    [1,H]
    x        = tanh(temp_dec + temp_enc)
    e        = x @ va^T                        [1,S,1]
    w        = softmax(mask ? e : -inf)        [1,S,1]
    ctx      = w^T @ enc                       [1,1,H]

Strategy: shard S across 8 cores (8192 rows each). Each core:
  - streams its enc shard from HBM once (f32), casts to bf16, transposes
    s<->h on TensorE (is_transpose matmuls; the DMA-xbar path costs ~1.2us
    of HWDGE descriptor-gen per 128x128 block on this runtime, measured),
  - temp_enc^T = WencT @ encT on TensorE (bf16, f32 accum in PSUM),
  - tanh+bias fused on ScalarE (bias = temp_dec as per-partition scalar,
    since the layout is transposed), output bf16,
  - energies via TensorE with the tanh block stationary and va moving
    (output lands as [128,1] columns -> natural [128,64] softmax layout),
  - exp WITHOUT max subtraction (|e| <= ||va||_1 ~ 18, safely in f32),
    masked, then partial context = sum_s exp(e_s) * enc_s accumulated in
    PSUM across all 64 s-tiles,
  - ONE AllReduce(add) of [partial_ctx(512) | partial_Z(1)] (2KB),
  - normalize: ctx = y/Z, w_s = exp(e_s)/Z.

All host<->device tensors stay contiguous; every partition-spread (dec, va,
temp_dec, mask, attention-weight store) goes through tiny TensorE transposes
or K=1 broadcast matmuls instead of 4-byte-strided DMA descriptors.

Exactness vs reference: softmax(e) is invariant to the max subtraction up to
f32 rounding; bf16 is only used for matmul operands (f32 accumulation).
"""

import numpy as np

import concourse.bass as bass
import concourse.mybir as mybir
import concourse.tile as tile
from concourse.bass_utils import run_bass_kernel_spmd
from concourse.masks import make_identity

NCORES = 8
H = 512
S = 65536
SLOC = S // NCORES  # 8192
P = 128
NST = SLOC // P  # 64 s-tiles of 128
CH = 4  # s-tiles per chunk (chunk = 512 sequence positions)
NCHUNK = NST // CH  # 16
NH = H // P  # 4 h-tiles
AR_N = 520  # 512 ctx + 1 Z + 7 pad (32B-aligned total)

F32 = mybir.dt.float32
BF16 = mybir.dt.bfloat16
AF = mybir.ActivationFunctionType

# ---------------------------------------------------------------------------
# walrus on this container rejects >1 sync-wait per instruction; hoist the
# excess onto same-engine NOPs placed immediately before the instruction.
_DEFAULT_LIMIT = 1


def _fix_wait_overflow(nc):
    eng_map = {
        mybir.EngineType.PE: nc.tensor,
        mybir.EngineType.DVE: nc.vector,
        mybir.EngineType.Activation: nc.scalar,
        mybir.EngineType.Pool: nc.gpsimd,
        mybir.EngineType.SP: nc.sync,
    }
    for bb in nc.main_func.blocks:
        insts = bb.instructions
        i = 0
        while i < len(insts):
            ins = insts[i]
            limit = _DEFAULT_LIMIT
            si = ins.sync_info
            waits = list(si.on_wait) if si and si.on_wait else []
            if len(waits) > limit:
                keep, extra = waits[:limit], waits[limit:]
                eng = eng_map[ins.engine]
                nops = []
                for j in range(len(extra)):
                    nop_ins = eng.nop(nofuse=True).ins
                    nop_ins.sync_info = type(si)(on_wait=[extra[j]], on_update=[])
                    for b2 in nc.main_func.blocks:
                        if nop_ins in b2.instructions:
                            b2.instructions.remove(nop_ins)
                            break
                    nops.append(nop_ins)
                ins.sync_info = type(si)(on_wait=keep, on_update=si.on_update)
                insts[i:i] = nops
                i += len(nops)
            i += 1
    return nc


# ---------------------------------------------------------------------------
def build_nc():
    nc = bass.Bass()

    enc = nc.declare_dram_parameter("enc", [SLOC, H], F32, isOutput=False)
    dec = nc.declare_dram_parameter("dec", [1, H], F32, isOutput=False)
    msk = nc.declare_dram_parameter("msk", [SLOC], F32, isOutput=False)
    wenc = nc.declare_dram_parameter("wenc", [H, H], F32, isOutput=False)
    wdec = nc.declare_dram_parameter("wdec", [H, H], F32, isOutput=False)
    va = nc.declare_dram_parameter("va", [1, H], F32, isOutput=False)
    attn_out = nc.declare_dram_parameter("attn_out", [SLOC], F32, isOutput=True)
    ctx_out = nc.declare_dram_parameter("ctx_out", [1, H], F32, isOutput=True)

    ar_in = nc.dram_tensor("ar_in", [1, AR_N], F32)
    ar_out = nc.dram_tensor("ar_out", [1, AR_N], F32, addr_space="Shared")

    with tile.TileContext(nc) as tc:
        with (
            tc.tile_pool(name="singles", bufs=1) as singles,
            tc.tile_pool(name="wsetup", bufs=1) as wsetup,
            tc.tile_pool(name="io", bufs=8) as io,
            tc.tile_pool(name="pt", bufs=4, space="PSUM") as pt_pool,
            tc.tile_pool(name="psum1", bufs=2, space="PSUM") as psum1,
            tc.tile_pool(name="pe", bufs=1, space="PSUM") as pe_pool,
            tc.tile_pool(name="psum_ctx", bufs=1, space="PSUM") as psum_ctx,
        ):
            # ---------------- setup ----------------
            id_bf = singles.tile([P, P], BF16)
            make_identity(nc, id_bf[:])
            id_f = singles.tile([P, P], F32)
            make_identity(nc, id_f[:])
            one1 = singles.tile([1, 1], F32)
            nc.vector.memset(one1[:], 1.0)
            ones_col = singles.tile([P, 1], F32)
            nc.vector.memset(ones_col[:], 1.0)
            ones_row = singles.tile([1, P], F32)
            nc.vector.memset(ones_row[:], 1.0)

            # weights: load f32, PE-transpose per 128-block, evac as bf16
            wencT = []  # [hi] -> [128 h, 512 o] bf16
            wdecT = []
            for name, wdram, dst in (("we", wenc, wencT), ("wd", wdec, wdecT)):
                w_f32 = []
                for oi in range(NH):
                    t = wsetup.tile([P, H], F32, tag=f"{name}f32_{oi}")
                    nc.sync.dma_start(t[:], wdram[oi * P : (oi + 1) * P, :])
                    w_f32.append(t)
                for hi in range(NH):
                    pw = pt_pool.tile([P, H], F32, tag="pt")
                    for oi in range(NH):
                        nc.tensor.transpose(
                            pw[:, oi * P : (oi + 1) * P],
                            w_f32[oi][:, hi * P : (hi + 1) * P],
                            id_f[:],
                        )
                    t = wsetup.tile([P, H], BF16, tag=f"{name}T_{hi}")
                    nc.vector.tensor_copy(t[:], pw[:])
                    dst.append(t)

            # dec / va: contiguous load + K=1 broadcast matmuls -> columns
            dec_nat = singles.tile([1, H], F32)
            va_nat = singles.tile([1, H], F32)
            nc.sync.dma_start(dec_nat[:], dec[:])
            nc.sync.dma_start(va_nat[:], va[:])
            dv_cols = []
            for cname, src in (("dec", dec_nat), ("va", va_nat)):
                pc = pe_pool.tile([P, NH], F32, tag="pe")
                for hi in range(NH):
                    nc.tensor.matmul(
                        pc[:, hi : hi + 1],
                        src[0:1, hi * P : (hi + 1) * P],
                        one1[:],
                        start=True,
                        stop=True,
                    )
                col = singles.tile([P, NH], BF16, tag=f"col_{cname}")
                nc.vector.tensor_copy(col[:], pc[:])
                dv_cols.append(col)
            dec_col, va_col = dv_cols

            # mask: contiguous [64,128] load, PE-transpose -> [128,64]
            mask_nat = singles.tile([NST, P], F32)
            nc.sync.dma_start(mask_nat[:], msk[:].rearrange("(j p) -> j p", p=P))
            pm = pt_pool.tile([P, NST], F32, tag="pt")
            nc.tensor.transpose(pm[:], mask_nat[:], id_f[:NST, :NST])
            mask_sb = singles.tile([P, NST], F32)
            nc.vector.tensor_copy(mask_sb[:], pm[:])

            # temp_dec^T columns: td[p, oi] = sum_h Wdec[oi*128+p, h] dec[h]
            td_psum = pe_pool.tile([P, NH], F32, tag="pe")
            for oi in range(NH):
                for hi in range(NH):
                    nc.tensor.matmul(
                        td_psum[:, oi : oi + 1],
                        wdecT[hi][:, oi * P : (oi + 1) * P],
                        dec_col[:, hi : hi + 1],
                        start=(hi == 0),
                        stop=(hi == NH - 1),
                    )
            td_col = singles.tile([P, NH], F32)
            nc.vector.tensor_copy(td_col[:], td_psum[:])

            # persistent softmax state
            expm = singles.tile([P, NST], F32)  # masked exp(e)
            w_bf16 = singles.tile([P, NST], BF16)  # bf16 copy for ctx matmul
            ctx_psum = psum_ctx.tile([1, H], F32)

            # ---------------- main loop ----------------
            for c in range(NCHUNK):
                s0 = c * CH * P
                enc_f32 = []
                for st in range(CH):
                    t = io.tile([P, H], F32, tag="enc_f32")
                    nc.sync.dma_start(t[:], enc[s0 + st * P : s0 + (st + 1) * P, :])
                    enc_f32.append(t)
                enc_bf = []
                for st in range(CH):
                    t = io.tile([P, H], BF16, tag="enc_bf")
                    nc.vector.tensor_copy(t[:], enc_f32[st][:])
                    enc_bf.append(t)
                # PE transpose: encT[hi][:, st*128:(st+1)*128] = enc_bf[st][:, hi]^T
                encT = []
                for hi in range(NH):
                    ptt = pt_pool.tile([P, CH * P], BF16, tag="pt")
                    for st in range(CH):
                        nc.tensor.transpose(
                            ptt[:, st * P : (st + 1) * P],
                            enc_bf[st][:, hi * P : (hi + 1) * P],
                            id_bf[:],
                        )
                    t = io.tile([P, CH * P], BF16, tag="encT")
                    nc.vector.tensor_copy(t[:], ptt[:])
                    encT.append(t)
                # temp_enc^T blocks + fused tanh
                x_bf = []
                for oi in range(NH):
                    p1 = psum1.tile([P, CH * P], F32, tag="psum1")
                    for hi in range(NH):
                        nc.tensor.matmul(
                            p1[:],
                            wencT[hi][:, oi * P : (oi + 1) * P],
                            encT[hi][:],
                            start=(hi == 0),
                            stop=(hi == NH - 1),
                        )
                    xt = io.tile([P, CH * P], BF16, tag="x_bf")
                    nc.scalar.activation(
                        xt[:], p1[:], AF.Tanh, bias=td_col[:, oi : oi + 1]
                    )
                    x_bf.append(xt)
                # energies into [128, CH] columns
                pe_t = pe_pool.tile([P, CH], F32, tag="pe")
                for st in range(CH):
                    for oi in range(NH):
                        nc.tensor.matmul(
                            pe_t[:, st : st + 1],
                            x_bf[oi][:, st * P : (st + 1) * P],
                            va_col[:, oi : oi + 1],
                            start=(oi == 0),
                            stop=(oi == NH - 1),
                        )
                cols = slice(c * CH, (c + 1) * CH)
                nc.scalar.activation(expm[:, cols], pe_t[:], AF.Exp)
                nc.vector.tensor_mul(expm[:, cols], expm[:, cols], mask_sb[:, cols])
                nc.vector.tensor_copy(w_bf16[:, cols], expm[:, cols])
                # partial context accumulation (k = s on partitions)
                for st in range(CH):
                    j = c * CH + st
                    nc.tensor.matmul(
                        ctx_psum[:],
                        w_bf16[:, j : j + 1],
                        enc_bf[st][:],
                        start=(j == 0),
                        stop=(j == NST - 1),
                        skip_group_check=True,
                    )

            # ---------------- tail ----------------
            zrow = singles.tile([P, 1], F32)
            nc.vector.reduce_sum(zrow[:], expm[:], axis=mybir.AxisListType.X)
            z_psum = pe_pool.tile([1, 1], F32, tag="pe")
            nc.tensor.matmul(z_psum[:], ones_col[:], zrow[:], start=True, stop=True)
            ar_sb = singles.tile([1, AR_N], F32)
            nc.vector.memset(ar_sb[:], 0.0)
            nc.vector.tensor_copy(ar_sb[0:1, 0:H], ctx_psum[:])
            nc.vector.tensor_copy(ar_sb[0:1, H : H + 1], z_psum[:])
            nc.sync.dma_start(ar_in[:], ar_sb[:])
            nc.gpsimd.collective_compute(
                "AllReduce",
                mybir.AluOpType.add,
                replica_groups=[list(range(NCORES))],
                ins=[ar_in[:]],
                outs=[ar_out[:]],
            )
            g = singles.tile([1, AR_N], F32)
            nc.sync.dma_start(g[:], ar_out[:])
            rz = singles.tile([1, 1], F32)
            nc.vector.reciprocal(rz[:], g[0:1, H : H + 1])
            ctx_sb = singles.tile([1, H], F32)
            nc.vector.tensor_scalar_mul(ctx_sb[:], g[0:1, 0:H], rz[:])
            nc.sync.dma_start(ctx_out[:], ctx_sb[:])
            # w = expm / Z, stored transposed for a contiguous write
            b_psum = pe_pool.tile([P, 1], F32, tag="pe")
            nc.tensor.matmul(b_psum[:], ones_row[:], rz[:], start=True, stop=True)
            rz_col = singles.tile([P, 1], F32)
            nc.vector.tensor_copy(rz_col[:], b_psum[:])
            w_f32 = singles.tile([P, NST], F32)
            nc.vector.tensor_scalar_mul(w_f32[:], expm[:], rz_col[:])
            pa = pt_pool.tile([NST, P], F32, tag="pt")
            nc.tensor.transpose(pa[:], w_f32[:], id_f[:])
            w_T = singles.tile([NST, P], F32)
            nc.vector.tensor_copy(w_T[:], pa[:])
            nc.sync.dma_start(attn_out[:].rearrange("(j p) -> j p", p=P), w_T[:])

    return _fix_wait_overflow(nc)


_NC = None


def _get_nc():
    global _NC
    if _NC is None:
        _NC = build_nc()
    return _NC


def kernel(encoder_output, decoder_hidden, attention_mask, Wenc_w, Wdec_w, va_w):
    enc = np.ascontiguousarray(np.asarray(encoder_output, dtype=np.float32))[0]
    dec = np.ascontiguousarray(np.asarray(decoder_hidden, dtype=np.float32))
    mask_f = np.asarray(attention_mask).astype(np.float32)
    wenc = np.ascontiguousarray(np.asarray(Wenc_w, dtype=np.float32))
    wdec = np.ascontiguousarray(np.asarray(Wdec_w, dtype=np.float32))
    va = np.ascontiguousarray(np.asarray(va_w, dtype=np.float32))

    in_maps = []
    for r in range(NCORES):
        sl = slice(r * SLOC, (r + 1) * SLOC)
        in_maps.append(
            {
                "enc": np.ascontiguousarray(enc[sl]),
                "dec": dec,
                "msk": np.ascontiguousarray(mask_f[sl]),
                "wenc": wenc,
                "wdec": wdec,
                "va": va,
            }
        )

    res = run_bass_kernel_spmd(_get_nc(), in_maps, list(range(NCORES)))

    attn = np.concatenate(
        [np.asarray(res.results[r]["attn_out"]).reshape(-1) for r in range(NCORES)]
    ).reshape(1, S, 1)
    ctx = np.asarray(res.results[0]["ctx_out"]).reshape(1, 1, H)
    return ctx, attn
